# revision 1
# baseline (speedup 1.0000x reference)
"""2-layer quantized-weight GRU on one NeuronCore (per-core batch shard).

Layout: hidden/gates on partitions, batch (B=32) on free dim. All h state
kept transposed [H, B] fp16, so the recurrence never transposes and h tiles
feed matmuls directly as moving operands.

Weights are fake-quantized (W ~ s*Q, Q integer); we store s*Q rounded to
fp16 as the stationary operand (FWL fast load, 1 cyc/row). PSUM holds true
gate pre-activations: the batched input-projection matmuls write chunk PSUM
tiles, per-step recurrent matmuls accumulate r,z in-place on top, and the
rz biases enter via a K=2 selector matmul. Gates run on ACT (sigmoid over
[c,2,B], tanh), n-path and h-update on DVE with stt fusion.

Software pipeline: layer-1 steps of chunk j interleave with layer-2 steps
of chunk j-1 so ACT/DVE/PE stay busy across the two recurrent chains.
"""

from contextlib import ExitStack

import concourse.mybir as mybir
import concourse.tile as tile
from concourse import bacc

F32 = mybir.dt.float32
F16 = mybir.dt.float16
AF = mybir.ActivationFunctionType
OP = mybir.AluOpType

H = 256          # hidden/input size
G = 3            # gates
C = 2            # 128-chunks of H
B = 32           # per-core batch
P = 128


def build(S=512, T=8, R=1):
    """Input tensors (per core):
      xk:   [128, 2*S*B] f16  x packed [p, (k, s*B+b)] (p,k index input dim)
      wq:   [128, 4*2*768] f16 scaled quantized weights as lhsT tiles,
            mats: Wih1, Whh1, Wih2, Whh2; [p=k-row, (mat, k, g*256+c*128+m)]
      brz:  [2, 2*2*128] f16  rz bias columns: [rz, (layer, c, m)]
      bsel: [2, 2*256] f16    selector rhs for rz bias matmul
      bn:   [128, 2*2*2] f32  n-gate biases [p, (layer, which, c)]
            which 0 = b_hh_n (inner), 1 = b_ih_n (outer)
      hin:  [128, 2*2*32] f16 initial h [p, (layer, c, B)]
    Output:
      hout: [128, 2*32] f32   final h2 transposed [p, (c, B)]
    """
    assert S % T == 0
    NCH = S // T
    TB = T * B

    nc = bacc.Bacc("TRN2", target_bir_lowering=False, debug=False, num_devices=8)

    xk = nc.dram_tensor("xk", [P, 2 * S * B], F16, kind="ExternalInput")
    wq = nc.dram_tensor("wq", [P, 4 * 2 * G * H], F16, kind="ExternalInput")
    brz = nc.dram_tensor("brz", [2, 2 * C * P], F16, kind="ExternalInput")
    bsel = nc.dram_tensor("bsel", [2, 2 * H], F16, kind="ExternalInput")
    bhn = nc.dram_tensor("bhn", [P, 2 * C * B], F32, kind="ExternalInput")
    bnin = nc.dram_tensor("bnin", [2, 2 * P], F16, kind="ExternalInput")
    hin = nc.dram_tensor("hin", [P, 2 * C * B], F16, kind="ExternalInput")
    hout = nc.dram_tensor("hout", [P, C * B], F16, kind="ExternalOutput")

    with tile.TileContext(nc) as tc, ExitStack() as ctx:
        const = ctx.enter_context(tc.tile_pool(name="const", bufs=1))
        xpool = ctx.enter_context(tc.tile_pool(name="xp", bufs=3))
        h1cpool = ctx.enter_context(tc.tile_pool(name="h1c", bufs=2))
        hpool = ctx.enter_context(tc.tile_pool(name="hp", bufs=3))
        small = ctx.enter_context(tc.tile_pool(name="sm", bufs=4))
        przpool = ctx.enter_context(tc.tile_pool(name="prz", bufs=1, space="PSUM"))
        pnpool = ctx.enter_context(tc.tile_pool(name="pn", bufs=1, space="PSUM"))
        pnhpool = ctx.enter_context(tc.tile_pool(name="pnh", bufs=2, space="PSUM"))

        # ---- load constants ----
        w_sb = const.tile([P, 4, 2, G * H], F16)
        nc.sync.dma_start(out=w_sb, in_=wq.rearrange("p (m k f) -> p m k f", m=4, k=2))
        brz_sb = const.tile([2, 2, C, P], F16)
        nc.sync.dma_start(out=brz_sb, in_=brz.rearrange("q (l c m) -> q l c m", l=2, c=C))
        bsel_sb = const.tile([2, 2 * H], F16)
        nc.sync.dma_start(out=bsel_sb, in_=bsel.ap())
        bhn_sb = const.tile([P, 2, C, B], F32)
        nc.sync.dma_start(out=bhn_sb, in_=bhn.rearrange("p (l c b) -> p l c b", l=2, c=C))
        bnin_sb = const.tile([2, 2, P], F16)
        nc.sync.dma_start(out=bnin_sb, in_=bnin.rearrange("q (l m) -> q l m", l=2))
        hin_sb = const.tile([P, 2, C, B], F16)
        nc.sync.dma_start(out=hin_sb, in_=hin.rearrange("p (l c b) -> p l c b", l=2, c=C))

        xk_r = xk.rearrange("p (k f) -> p k f", k=2)

        def wslice(m, k, c, g):
            return w_sb[:, m, k, g * H + c * P: g * H + c * P + P]

        # rolling state (APs into fp16 tiles)
        h_prev = [hin_sb[:, l, :, :] for l in (0, 1)]

        def gi_chunk_mms(l, prz, pn, rhs_k):
            m = 0 if l == 0 else 2
            for c in range(C):
                nc.tensor.matmul(
                    prz[:, c, :, :].rearrange("p g f -> p (g f)"),
                    brz_sb[:, l, c, :], bsel_sb[:, :],
                    start=True, stop=False, skip_group_check=True)
            nc.tensor.matmul(
                pn[:, :, :].rearrange("p c f -> p (c f)"),
                bnin_sb[:, l, :], bsel_sb[:, :],
                start=True, stop=False, skip_group_check=True)
            for c in range(C):
                for g in (0, 1):
                    for k in range(2):
                        nc.tensor.matmul(
                            prz[:, c, g, :], wslice(m, k, c, g), rhs_k(k),
                            start=False, stop=False, skip_group_check=True)
            for c in range(C):
                for k in range(2):
                    nc.tensor.matmul(
                        pn[:, c, :], wslice(m, k, c, 2), rhs_k(k),
                        start=False, stop=False, skip_group_check=True)

        def mm_part(l, t, prz, pn):
            """Recurrent matmuls + the off-chain pnh bias add for step t."""
            m = 1 if l == 0 else 3
            hp = h_prev[l]
            pnh = pnhpool.tile([P, C, B], F32, tag="pnh")
            with tc.tile_critical():
                for c in range(C):
                    for g in (0, 1):
                        for k in range(2):
                            nc.tensor.matmul(
                                prz[:, c, g, t * B:(t + 1) * B],
                                wslice(m, k, c, g), hp[:, k, :],
                                start=False, stop=(k == 1), skip_group_check=True)
                for c in range(C):
                    for k in range(2):
                        nc.tensor.matmul(
                            pnh[:, c, :], wslice(m, k, c, 2), hp[:, k, :],
                            start=(c == 0 and k == 0), stop=(k == 1),
                            skip_group_check=True)
            # off-chain: u = pnh + b_hh_n (runs while sigmoid is on ACT)
            u = small.tile([P, C, B], F32, tag="u")
            nc.vector.tensor_add(u, pnh, bhn_sb[:, l, :, :])
            return hp, u

        def gate_part(l, t, prz, pn, hp, u, h1chunk_slot):
            rz = small.tile([P, C, 2, B], F32, tag="rz")
            nc.scalar.activation(rz, prz[:, :, :, t * B:(t + 1) * B], AF.Sigmoid)
            # zbar = 1 - z via sigmoid(-preact_z), off the recurrence cycle
            zb = small.tile([P, C, B], F32, tag="zb")
            nc.scalar.activation(zb, prz[:, :, 1, t * B:(t + 1) * B],
                                 AF.Sigmoid, scale=-1.0)
            # n = tanh((pn + b_ih_n) + r * u)
            w = small.tile([P, C, B], F32, tag="w")
            nc.vector.tensor_mul(w, u, rz[:, :, 0, :])
            nc.vector.tensor_add(w, w, pn[:, :, t * B:(t + 1) * B])
            # off-cycle filler: zh = z * h
            zh = small.tile([P, C, B], F32, tag="zh")
            nc.vector.tensor_mul(zh, rz[:, :, 1, :], hp)
            n = small.tile([P, C, B], F32, tag="n")
            nc.scalar.activation(n, w, AF.Tanh)
            # h' = n*zbar + zh
            d = small.tile([P, C, B], F32, tag="d")
            nc.vector.tensor_mul(d, n, zb)
            if l == 0:
                hnew = h1chunk_slot[:, :, t * B:(t + 1) * B]
            else:
                hnew = hpool.tile([P, C, B], F16, tag="h2")
            nc.vector.tensor_add(hnew, d, zh)
            h_prev[l] = hnew

        # software pipeline: L1 of chunk j interleaved with L2 of chunk j-1
        def l1_chunk(j):
            xt = xpool.tile([P, 2, TB], F16, tag="x")
            nc.sync.dma_start(out=xt, in_=xk_r[:, :, j * TB:(j + 1) * TB])
            prz1 = przpool.tile([P, C, 2, TB], F32, tag="prz1")
            pn1 = pnpool.tile([P, C, TB], F32, tag="pn1")
            gi_chunk_mms(0, prz1, pn1, lambda k: xt[:, k, :])
            h1chunk = h1cpool.tile([P, C, TB], F16, tag="h1c")
            return prz1, pn1, h1chunk

        def l2_chunk(h1chunk):
            prz2 = przpool.tile([P, C, 2, TB], F32, tag="prz2")
            pn2 = pnpool.tile([P, C, TB], F32, tag="pn2")
            gi_chunk_mms(1, prz2, pn2, lambda k: h1chunk[:, k, :])
            return prz2, pn2

        for _rep in range(R):
          if _rep:
            h_prev[0] = hin_sb[:, 0, :, :]
            h_prev[1] = hin_sb[:, 1, :, :]
          cur1 = l1_chunk(0)
          for t in range(T):
            hp, u = mm_part(0, t, cur1[0], cur1[1])
            gate_part(0, t, cur1[0], cur1[1], hp, u, cur1[2])
          prev1 = cur1
          pend2 = None
          for j in range(1, NCH):
            cur2 = l2_chunk(prev1[2])
            cur1 = l1_chunk(j)
            for t in range(T):
                st1 = (t, cur1) + mm_part(0, t, cur1[0], cur1[1])
                if pend2 is not None:
                    t2, c2, hp2, u2 = pend2
                    gate_part(1, t2, c2[0], c2[1], hp2, u2, None)
                pend2 = (t, cur2) + mm_part(1, t, cur2[0], cur2[1])
                gate_part(0, st1[0], st1[1][0], st1[1][1], st1[2], st1[3], cur1[2])
            prev1 = cur1
          cur2 = l2_chunk(prev1[2])
          for t in range(T):
            if pend2 is not None:
                t2, c2, hp2, u2 = pend2
                gate_part(1, t2, c2[0], c2[1], hp2, u2, None)
            pend2 = (t, cur2) + mm_part(1, t, cur2[0], cur2[1])
          t2, c2, hp2, u2 = pend2
          gate_part(1, t2, c2[0], c2[1], hp2, u2, None)

        nc.sync.dma_start(out=hout.rearrange("p (c b) -> p c b", c=C), in_=h_prev[1])

    nc.compile()
    return nc


def quantize(w):
    """Match reference fake_quant in float32."""
    import numpy as np
    w = np.asarray(w, np.float32)
    scale = np.float32(np.max(np.abs(w)) / np.float32(127.0))
    q = np.round((w / scale).astype(np.float32))
    return q.astype(np.float32), scale


def host_pack(inputs, S=512):
    """Build per-core in_maps from full inputs dict. Returns in_maps."""
    import numpy as np

    x = np.asarray(inputs["x"], np.float32)          # [S, 256, 256]
    NB = x.shape[1]
    ncores = NB // B

    wfq = []
    for key in ("w_ih1", "w_hh1", "w_ih2", "w_hh2"):
        q, s = quantize(inputs[key])
        wfq.append((q * s).astype(np.float32))

    def wpack(wf):
        return np.ascontiguousarray(wf.T).reshape(2, P, G * H)
    wstack = np.stack([wpack(wf) for wf in wfq])     # [4,2,128,768]
    wqarr = np.ascontiguousarray(wstack.transpose(2, 0, 1, 3)).reshape(
        P, 4 * 2 * G * H).astype(np.float16)

    brz = np.zeros((2, 2, C, P), np.float32)
    bhn = np.zeros((P, 2, C, B), np.float32)
    bnin = np.zeros((2, 2, P), np.float32)
    for l, (bi, bh) in enumerate((
            (inputs["b_ih1"], inputs["b_hh1"]),
            (inputs["b_ih2"], inputs["b_hh2"]))):
        bi = np.asarray(bi, np.float32)
        bh = np.asarray(bh, np.float32)
        for g in (0, 1):
            brz[g, l] = (bi[g * H:(g + 1) * H] + bh[g * H:(g + 1) * H]).reshape(C, P)
        bhn[:, l] = np.repeat((bh[2 * H:]).reshape(C, P).T[:, :, None], B, axis=2)
        bnin[:, l] = (bi[2 * H:]).reshape(C, P)
    brz = brz.reshape(2, 2 * C * P).astype(np.float16)
    bhn = bhn.reshape(P, 2 * C * B)
    bnin = bnin.reshape(2, 2 * P).astype(np.float16)

    bsel = np.zeros((2, 2 * H), np.float16)
    bsel[0, :H] = 1.0
    bsel[1, H:] = 1.0

    h1 = np.asarray(inputs["h1"], np.float32)
    h2 = np.asarray(inputs["h2"], np.float32)

    in_maps = []
    for cidx in range(ncores):
        sl = slice(cidx * B, (cidx + 1) * B)
        xs = np.ascontiguousarray(x[:, sl, :].transpose(2, 0, 1)).reshape(2, P, S * B)
        xs = np.ascontiguousarray(xs.transpose(1, 0, 2)).reshape(P, 2 * S * B)
        xs = xs.astype(np.float16)

        hinit = np.zeros((P, 2, C, B), np.float16)
        for l, h in enumerate((h1, h2)):
            hT = np.ascontiguousarray(h[sl].T)          # [256, B]
            hinit[:, l] = hT.reshape(C, P, B).transpose(1, 0, 2).astype(np.float16)
        in_maps.append({
            "xk": xs, "wq": wqarr, "brz": brz, "bsel": bsel,
            "bhn": bhn, "bnin": bnin, "hin": hinit.reshape(P, 2 * C * B),
        })
    return in_maps


def host_unpack(results):
    """results: list of per-core {'hout': [128, 64]} -> full [NB, 256]."""
    import numpy as np
    outs = []
    for r in results:
        ht = r["hout"].astype(np.float32).reshape(P, C, B)  # [p, c, b]
        h = ht.transpose(1, 0, 2).reshape(H, B).T       # [B, 256]
        outs.append(h)
    return np.ascontiguousarray(np.concatenate(outs, axis=0))


# ---------------------------------------------------------------------------
# Harness entry point: full (unsharded) inputs -> full output.
# ---------------------------------------------------------------------------

_SEQ = 512


def kernel(**inputs):
    """2-layer fake-quantized GRU encoder. Shards batch 8 ways across
    NeuronCores, runs the Bass kernel, gathers final h2 [256, 256] f32."""
    import numpy as np
    from concourse.bass_utils import run_bass_kernel_spmd

    in_maps = host_pack(inputs, S=_SEQ)
    nc = build(S=_SEQ, T=8)
    res = run_bass_kernel_spmd(nc, in_maps, core_ids=list(range(len(in_maps))))
    out = host_unpack(res.results)
    return out.astype(np.float32)

